# revision 11
# baseline (speedup 1.0000x reference)
"""CrossAttention (B=4, N=M=2048, C=1024, H=16, D=64) on 8 TRN2 cores.

Sharding: core = 2*b + g  (b = batch 0..3, g = head-half 0..1, 8 heads each).
Each core computes attention for its 8 heads and a partial (full-width)
output projection over its 512 local channels; the host sums the two
partials per batch and transposes back.

Device layout notes:
  - All activations live transposed (channels on partitions) so every
    matmul has its contraction on the partition axis with no on-chip
    transposes.  The host feeds query/key/value pre-transposed.
  - scoresT = K_h^T-stationary x qT-moving -> (m on partitions, n free);
    softmax denominator comes free from a ones-column appended to V
    (M=65 AV matmul, row 64 of the accumulator is sum_m exp(s)).
  - exp on the scalar engine (psum->sbuf, width 1024, cast to bf16).
  - Normalization: DVE reciprocal of the denominator row + GPSIMD
    partition-broadcast + DVE multiply into xT.
"""

from contextlib import ExitStack

import ml_dtypes
import numpy as np

import concourse.bass as bass
import concourse.mybir as mybir
import concourse.tile as tile
from concourse import bacc, library_config
from concourse.bass_utils import run_bass_kernel_spmd

dt = mybir.dt
AF = mybir.ActivationFunctionType

# Problem dims (hardcoded; must match the harness inputs).
B, N, M, C, H = 4, 2048, 2048, 1024, 16
D = C // H            # 64
SCALE = D ** -0.5     # 0.125 (exact)
CL = C // 2           # 512 channels per core (8 heads)
HL = H // 2           # 8 local heads
P = 128
CT = C // P           # 8 input-channel tiles
DT = CL // P          # 4 local-channel tiles
MT = M // P           # 16 key tiles
NCH = 512             # moving free-dim chunk (fp32r max)
NCHUNKS = N // NCH    # 4
EXPW = 1024           # exp width (2 psum banks)
VA = D + 1            # 65: v columns + ones column

F32 = dt.float32
F32R = dt.float32r
BF16 = dt.bfloat16


def build_program(reps: int = 1) -> bass.Bass:
    """reps>1 repeats the whole body for timing (wall-time delta isolates
    device time from host/transfer overhead)."""
    nc = bacc.Bacc()
    nc.gpsimd.load_library(library_config.attn)

    qTin = nc.declare_dram_parameter("qTin", [C, N], F32R, isOutput=False)
    kTin = nc.declare_dram_parameter("kTin", [C, M], F32R, isOutput=False)
    vTin = nc.declare_dram_parameter("vTin", [C, M], F32R, isOutput=False)
    wq = nc.declare_dram_parameter("wq", [C, CL], F32R, isOutput=False)
    wk = nc.declare_dram_parameter("wk", [C, CL], F32R, isOutput=False)
    wv = nc.declare_dram_parameter("wv", [C, CL], F32R, isOutput=False)
    wp = nc.declare_dram_parameter("wp", [CL, C], dt.bfloat16, isOutput=False)
    bp = nc.declare_dram_parameter("bp", [C], F32, isOutput=False)
    out = nc.declare_dram_parameter("out", [C, N], F32, isOutput=True)

    with tile.TileContext(nc) as tc:
      for _rep in range(reps):
       with ExitStack() as ctx:
        # ---- persistent sbuf tensors -------------------------------------
        const_pool = ctx.enter_context(tc.tile_pool(name="consts", bufs=1))
        wp_sb = const_pool.tile([P, DT * C], BF16)
        bp_sb = const_pool.tile([P, CT], F32)
        qT_sb = const_pool.tile([P, DT * N], F32R)   # local q, transposed
        kT_sb = const_pool.tile([P, DT * M], F32R)   # local k, transposed
        va_sb = const_pool.tile([P, MT * HL * VA], BF16)  # 8x(64 v + 1) per m-tile
        xT_sb = const_pool.tile([P, DT * N], BF16)  # attention out, transposed

        nc.sync.dma_start(
            out=wp_sb[:].rearrange("p (ct d) -> p ct d", d=C),
            in_=wp[:, :].rearrange("(ct p) d -> p ct d", p=P),
        )
        nc.sync.dma_start(
            out=bp_sb[:],
            in_=bp[:].rearrange("(t p) -> p t", p=P),
        )

        # ---- phase 1: projections (weights + input streams are scoped) --
        with tc.tile_pool(name="wqkv", bufs=1) as w_pool, \
             tc.tile_pool(name="inT", bufs=16) as in_pool, \
             tc.tile_pool(name="ps", bufs=2, space="PSUM") as ps_pool0:
            # weight tiles: [c-part, (ct, d)] so lhsT/rhs slices are contiguous
            wq_sb = w_pool.tile([P, CT * CL], F32R)
            wk_sb = w_pool.tile([P, CT * CL], F32R)
            wv_sb = w_pool.tile([P, CT * CL], F32R)
            for w_dram, w_sb in ((wq, wq_sb), (wk, wk_sb), (wv, wv_sb)):
                nc.sync.dma_start(
                    out=w_sb[:].rearrange("p (ct d) -> p ct d", d=CL),
                    in_=w_dram[:, :].rearrange("(ct p) d -> p ct d", p=P),
                )

            def load_chunk(src, ch):
                tiles = []
                for ct in range(CT):
                    t = in_pool.tile([P, NCH], F32R, tag="inT", name=f"i{ct}")
                    nc.sync.dma_start(
                        out=t[:],
                        in_=src[ct * P:(ct + 1) * P, ch * NCH:(ch + 1) * NCH],
                    )
                    tiles.append(t)
                return tiles

            # qT/kT: dst[d, n] (d on partitions): lhsT = wT tile, rhs = inT
            for src, w_sb, dst_sb in ((qTin, wq_sb, qT_sb),
                                      (kTin, wk_sb, kT_sb)):
                for ch in range(NCHUNKS):
                    in_tiles = load_chunk(src, ch)
                    for j in range(DT):
                        acc = ps_pool0.tile([P, EXPW], F32, tag="big", name="prj")
                        for ct in range(CT):
                            nc.tensor.matmul(
                                acc[:, :NCH],
                                w_sb[:, ct * CL + j * P: ct * CL + (j + 1) * P],
                                in_tiles[ct][:],
                                start=(ct == 0),
                                stop=(ct == CT - 1),
                            )
                        nc.vector.tensor_copy(
                            dst_sb[:, j * N + ch * NCH: j * N + (ch + 1) * NCH],
                            acc[:, :NCH],
                        )

            # v natural (m on partitions): lhsT = vTin slice, rhs = wvT
            for mg in range(4):                 # m-groups of 4 m-tiles
                in_tiles = load_chunk(vTin, mg)
                for mi in range(4):
                    mt = mg * 4 + mi
                    acc = ps_pool0.tile([P, EXPW], F32, tag="big", name="prv")
                    for ct in range(CT):
                        nc.tensor.matmul(
                            acc[:, :CL],
                            in_tiles[ct][:, mi * P:(mi + 1) * P],
                            wv_sb[:, ct * CL:(ct + 1) * CL],
                            start=(ct == 0),
                            stop=(ct == CT - 1),
                        )
                    blk3 = va_sb[:, mt * HL * VA:(mt + 1) * HL * VA].rearrange(
                        "p (h e) -> p h e", e=VA)
                    nc.vector.tensor_copy(
                        blk3[:, :, :D],
                        acc[:, :CL].rearrange("p (h d) -> p h d", d=D),
                    )
                    nc.vector.memset(blk3[:, :, D:VA], 1.0)

        # ---- phase 2/3 pools (reuse the freed phase-1 space) -------------
        ps_pool = ctx.enter_context(tc.tile_pool(name="ps2", bufs=2, space="PSUM"))
        av_pool = ctx.enter_context(tc.tile_pool(name="av", bufs=4, space="PSUM"))
        pt_pool = ctx.enter_context(tc.tile_pool(name="pt", bufs=6))
        sm_pool = ctx.enter_context(tc.tile_pool(name="sm", bufs=4))
        ob_pool = ctx.enter_context(tc.tile_pool(name="ob", bufs=4))

        # ---- phase 2: attention per local head --------------------------
        for h in range(HL):
            j = h // 2
            roff = (h % 2) * D  # partition offset of this head inside tile j

            avs = [
                av_pool.tile([P, NCH], F32, tag="av", name=f"av{h}_{c}")
                for c in range(NCHUNKS)
            ]
            for mt in range(MT):
                for nh in range(2):
                    sc = ps_pool.tile([P, EXPW], F32, tag="big", name="sc")
                    for nn in range(2):
                        nc.tensor.matmul(
                            sc[:, nn * NCH:(nn + 1) * NCH],
                            kT_sb[roff:roff + D,
                                    j * M + mt * P: j * M + (mt + 1) * P],
                            qT_sb[roff:roff + D,
                                    j * N + nh * EXPW + nn * NCH:
                                    j * N + nh * EXPW + (nn + 1) * NCH],
                            start=True,
                            stop=True,
                        )
                    pt = pt_pool.tile([P, EXPW], BF16, tag="pt", name="pt")
                    nc.scalar.activation(pt[:], sc[:], AF.Exp)
                    for nn in range(2):
                        c = nh * 2 + nn
                        nc.tensor.matmul(
                            avs[c][:VA, :],
                            va_sb[:, mt * HL * VA + h * VA:
                                  mt * HL * VA + (h + 1) * VA],
                            pt[:, nn * NCH:(nn + 1) * NCH],
                            start=(mt == 0),
                            stop=(mt == MT - 1),
                        )

            for c in range(NCHUNKS):
                rc = sm_pool.tile([1, NCH], F32, tag="rc", name="rc")
                nc.vector.reciprocal(rc[:], avs[c][D:VA, :])
                bc = sm_pool.tile([D, NCH], F32, tag="bc", name="bc")
                nc.gpsimd.partition_broadcast(bc[:], rc[:])
                nc.vector.tensor_mul(
                    xT_sb[roff:roff + D,
                          j * N + c * NCH: j * N + (c + 1) * NCH],
                    avs[c][:D, :],
                    bc[:],
                )

        # ---- phase 3: output projection (partial over local channels) ---
        for mt8 in range(CT):
            for ch in range(NCHUNKS):
                acc = ps_pool.tile([P, EXPW], F32, tag="big", name="po")
                for ct in range(DT):
                    nc.tensor.matmul(
                        acc[:, :NCH],
                        wp_sb[:, ct * C + mt8 * P: ct * C + (mt8 + 1) * P],
                        xT_sb[:, ct * N + ch * NCH: ct * N + (ch + 1) * NCH],
                        start=(ct == 0),
                        stop=(ct == DT - 1),
                    )
                ob = ob_pool.tile([P, NCH], F32, tag="ob", name="ob")
                nc.vector.tensor_scalar_add(ob[:], acc[:, :NCH],
                                            bp_sb[:, mt8:mt8 + 1])
                nc.sync.dma_start(
                    out=out[mt8 * P:(mt8 + 1) * P, ch * NCH:(ch + 1) * NCH],
                    in_=ob[:],
                )

    nc.compile()
    return nc


_NC_CACHE = {}


def _get_program(reps: int = 1):
    if reps not in _NC_CACHE:
        _NC_CACHE[reps] = build_program(reps)
    return _NC_CACHE[reps]


def make_in_maps(query, key, value, Wq, Wk, Wv, Wp, bp):
    query = np.asarray(query, dtype=np.float32)
    key = np.asarray(key, dtype=np.float32)
    value = np.asarray(value, dtype=np.float32)
    Wq = np.asarray(Wq, dtype=np.float32)
    Wk = np.asarray(Wk, dtype=np.float32)
    Wv = np.asarray(Wv, dtype=np.float32)
    Wp = np.asarray(Wp, dtype=np.float32)
    bp = np.asarray(bp, dtype=np.float32)

    wqT = np.ascontiguousarray(Wq.T) * np.float32(SCALE)  # (C, C)
    wkT = np.ascontiguousarray(Wk.T)
    wvT = np.ascontiguousarray(Wv.T)
    wpT = np.ascontiguousarray(Wp.T).astype(ml_dtypes.bfloat16)  # (C, C)
    zeros_bp = np.zeros_like(bp)

    in_maps = []
    for core in range(8):
        b, g = divmod(core, 2)
        sl = slice(g * CL, (g + 1) * CL)
        in_maps.append({
            "qTin": np.ascontiguousarray(query[b].T),
            "kTin": np.ascontiguousarray(key[b].T),
            "vTin": np.ascontiguousarray(value[b].T),
            "wq": np.ascontiguousarray(wqT[:, sl]),
            "wk": np.ascontiguousarray(wkT[:, sl]),
            "wv": np.ascontiguousarray(wvT[:, sl]),
            "wp": np.ascontiguousarray(wpT[sl, :]),
            "bp": bp if g == 0 else zeros_bp,
        })
    return in_maps


def combine_outputs(results):
    out = np.empty((B, N, C), dtype=np.float32)
    for b in range(B):
        part = results[2 * b]["out"] + results[2 * b + 1]["out"]  # (C, N)
        out[b] = part.T
    return out


def kernel(**inputs) -> np.ndarray:
    nc = _get_program()
    in_maps = make_in_maps(**inputs)
    res = run_bass_kernel_spmd(nc, in_maps, list(range(8)))
    return combine_outputs(res.results)


if __name__ == "__main__":
    nc = _get_program()
    print("program built ok")
